# revision 1
# baseline (speedup 1.0000x reference)
"""Trainium2 Bass kernel for nn_AdditiveAttention (B=32, NQ=1, NK=4096, D=512, H=256).

Data-parallel over 8 NeuronCores: each core owns 4 batches. Per core:
  kprojT[h, t] = sum_d W_k[d, h] * keys[b, t, d]      (PE, fp16, W_k stationary)
  featT        = tanh(kprojT + qproj_b)               (ACT, bias fused, fp16 out)
  scores[t]    = sum_h w_v[h] * featT[h, t]           (PE matvec, fp16)
  out[b, t]    = softmax_t(scores) * values[b, t]     (exp straight from PSUM with
                                                       fused partial sums; scores
                                                       are O(4) so no max-subtract)

The keys shard is handed to the device pre-transposed ([4, 512, 4096]) and
pre-cast to fp16 (the kernel's compute precision) so the contraction dim lands
on SBUF partitions. Each batch's keys load is a single 3D-access-pattern DMA
(the ~0.6us per-DMA issue cost on the sync sequencer serializes, so fewer,
bigger DMAs win). A few self-matmuls on W_q at the start warm the PE HAM
clock-gate to 2.4 GHz before the real matmul stream begins.
"""

import numpy as np
import ml_dtypes

N_CORES = 8
B, NQ, NK, D, H = 32, 1, 4096, 512, 256
B_LOC = B // N_CORES  # 4 batches per core
KT = D // 128         # 4 contraction tiles
HT = H // 128         # 2 hidden tiles
TOKC = 512            # matvec chunk (= one PSUM bank of f32)
TOKP = 1024           # kproj/tanh chunk (2 PSUM banks)
NCP = NK // TOKP      # 4 kproj chunks per batch
QTOK = NK // 4        # batch-0 quarter width (ramp)
N_WARM = 14           # HAM warmup matmuls (bridge until keys arrive)


def _install_profile_hook():
    """Make trace=True / BASS_TRACE=1 usable when the image's antenv lacks
    axon_hooks (degrades silently if anything is missing)."""
    try:
        from antenv import axon_hooks  # noqa: F401
        return
    except ImportError:
        pass
    try:
        import sys
        import types

        import antenv
        from trn_agent_boot.trn_boot import _ntff_profile_via_ctypes

        mod = types.ModuleType("antenv.axon_hooks")
        mod._h = None
        mod.set_axon_ntff_profile_hook = lambda h: setattr(mod, "_h", h)
        mod.get_axon_ntff_profile_hook = lambda: mod._h
        antenv.axon_hooks = mod
        sys.modules["antenv.axon_hooks"] = mod
        mod._h = _ntff_profile_via_ctypes("/opt/axon/libaxon_pjrt.so")
    except Exception:
        pass


def build_nc():
    import concourse.tile as tile
    from concourse import bacc, mybir
    from concourse.tile_rust import add_dep_helper

    f32 = mybir.dt.float32
    f16 = mybir.dt.bfloat16  # bf16 streams 1 row/cycle on PE; fp16 measured ~1.2x slower
    Act = mybir.ActivationFunctionType
    AX = mybir.AxisListType.X

    nc = bacc.Bacc("TRN2", target_bir_lowering=False, debug=False,
                   num_devices=N_CORES)

    keysT_ext = nc.dram_tensor("keysT", [B_LOC, D, NK], f16, kind="ExternalInput")
    qT_ext = nc.dram_tensor("queriesT", [128, KT * B_LOC], f32, kind="ExternalInput")
    vals_ext = nc.dram_tensor("vals", [B_LOC, NK], f32, kind="ExternalInput")
    wk_ext = nc.dram_tensor("wk", [128, KT * H], f16, kind="ExternalInput")
    wq_ext = nc.dram_tensor("wq", [128, KT * H], f32, kind="ExternalInput")
    wv_ext = nc.dram_tensor("wv", [128, B_LOC * HT * 128], f16, kind="ExternalInput")
    out_ext = nc.dram_tensor("out", [B_LOC, NK], f32, kind="ExternalOutput")

    # [B_LOC, D, NK] viewed so one DMA can pull [128 part, KT, ntok]
    keys3d = keysT_ext.ap().rearrange("b (k p) n -> b k p n", p=128)

    with tile.TileContext(nc) as tc:
        with (
            tc.tile_pool(name="keys", bufs=3) as keys_pool,
            tc.tile_pool(name="keys0", bufs=4) as keys0_pool,
            tc.tile_pool(name="feat", bufs=8) as feat_pool,
            tc.tile_pool(name="static", bufs=1) as st,
            tc.tile_pool(name="kp", bufs=6, space="PSUM") as kp_pool,
            tc.tile_pool(name="sc", bufs=2, space="PSUM") as sc_pool,
        ):
            # ---- HAM warmup on memset data: PE activity needs no DMA, so
            # the clock-gate reaches 8/8 before the first real matmul ----
            wtile = st.tile([128, H], f32, tag="warm_in")
            nc.vector.memset(wtile[:], 1.0)
            warm_ps = sc_pool.tile([128, H], f32, tag="sc")
            for w in range(N_WARM):
                nc.tensor.matmul(warm_ps[:], wtile[:, 0:128], wtile[:],
                                 start=(w == 0), stop=(w == N_WARM - 1))
            warm_out = st.tile([128, 1], f32, tag="warm")
            nc.vector.reduce_max(warm_out[:], warm_ps[:], axis=AX)
            # dummy tanh: forces the exp_and_others ACT table load (~2.7us)
            # to happen during the ramp instead of before the first real tanh
            dummy_sb = st.tile([128, 1], f32, tag="dummy")
            nc.scalar.activation(dummy_sb[:], wtile[:, 0:1], Act.Tanh)

            # ---- loads: W_k and batch-0 keys first (gate the first real
            # matmuls), then the q-side, then the rest of the keys ----
            wk_sb = st.tile([128, KT, H], f16, tag="wk")
            nc.sync.dma_start(wk_sb[:], wk_ext.ap())
            kt_tiles = {}
            # batch-0 quarter 0 right behind W_k on the sync FIFO so the
            # first kproj group never waits (the FIFO completes in order;
            # parking it behind wq/qT/wv cost a ~1.4us PE idle that tripped
            # the HAM re-throttle for ~5us of half-clock matmuls)
            t = keys0_pool.tile([128, KT, QTOK], f16, tag="kt0")
            nc.sync.dma_start(t[:], keys3d[0, :, :, 0:QTOK]
                              .rearrange("k p n -> p k n"))
            kt_tiles[(0, 0)] = t
            wq_sb = st.tile([128, KT, H], f32, tag="wq")
            nc.sync.dma_start(wq_sb[:], wq_ext.ap())
            qin_sb = st.tile([128, KT, B_LOC], f32, tag="qin")
            nc.sync.dma_start(qin_sb[:], qT_ext.ap())
            # w_v padded to full 128-col stationaries (one per (b, h), the
            # vector at column 32*b, zeros elsewhere): an M=1 stationary was
            # breaking the LDWEIGHTS double-buffer cadence (+~280ns/chunk)
            wv_sb = st.tile([128, B_LOC, HT, 128], f16, tag="wv")
            nc.sync.dma_start(wv_sb[:], wv_ext.ap())
            for q in range(1, 4):
                t = keys0_pool.tile([128, KT, QTOK], f16, tag="kt0")
                nc.sync.dma_start(t[:], keys3d[0, :, :, q * QTOK:(q + 1) * QTOK]
                                  .rearrange("k p n -> p k n"))
                kt_tiles[(0, q)] = t
            # per-batch softmax rows live at partition 32*b (engine ops need
            # 32-aligned base partitions); vals/out ride the scalar HWDGE
            # queue so they never sit behind the big keys DMAs
            vals_sb = st.tile([128, NK], f32, tag="vals")
            for b in range(B_LOC):
                nc.scalar.dma_start(vals_sb[32 * b:32 * b + 1, :],
                                    vals_ext[b:b + 1, :])
            for b in range(1, B_LOC):
                t = keys_pool.tile([128, KT, NK], f16, tag="kt")
                nc.sync.dma_start(t[:], keys3d[b].rearrange("k p n -> p k n"))
                kt_tiles[b] = t

            # ---- qproj (f32, exact): qbias[h][:, b] = (queries @ W_q)^T ----
            qbias_sb = st.tile([128, HT, B_LOC], f32, tag="qbias")
            for h in range(HT):
                qp = sc_pool.tile([128, B_LOC], f32, tag="sc")
                for k in range(KT):
                    nc.tensor.matmul(
                        qp[:],
                        wq_sb[:, k, h * 128:(h + 1) * 128],
                        qin_sb[:, k, :],
                        start=(k == 0), stop=(k == KT - 1),
                    )
                nc.vector.tensor_copy(qbias_sb[:, h, :], qp[:])

            # ---- per-batch softmax state (row 32*b per batch) ----
            esc_sb = st.tile([128, NK], f32, tag="esc")       # exp(scores)*vals
            psum_sb = st.tile([128, NK // TOKC], f32, tag="psums")
            ssum_sb = st.tile([128, 1], f32, tag="ssum")
            recip_sb = st.tile([128, 1], f32, tag="recip")

            for b in range(B_LOC):
                r = 32 * b

                def ksrc(c0):
                    """keys AP maker for tokens starting at c0 of this batch."""
                    if b == 0:
                        q = c0 // QTOK
                        t = kt_tiles[(0, q)]
                        o = c0 - q * QTOK
                        return lambda k, j: t[:, k, o + j * TOKC:o + (j + 1) * TOKC]
                    t = kt_tiles[b]
                    return lambda k, j: t[:, k, c0 + j * TOKC:c0 + (j + 1) * TOKC]

                for c in range(NK // TOKC):
                    cp, j = c // 2, c % 2
                    src = ksrc(cp * TOKP)
                    fts = []
                    for h in range(HT):
                        ps = kp_pool.tile([128, TOKC], f32, tag="ps")
                        for k in range(KT):
                            nc.tensor.matmul(
                                ps[:],
                                wk_sb[:, k, h * 128:(h + 1) * 128],
                                src(k, j),
                                start=(k == 0), stop=(k == KT - 1),
                            )
                        ft = feat_pool.tile([128, TOKC], f16, tag="ft")
                        nc.scalar.activation(ft[:], ps[:], Act.Tanh,
                                             bias=qbias_sb[:, h, b:b + 1])
                        fts.append(ft)
                    sc = sc_pool.tile([128, TOKC], f32, tag="sc")
                    for h in range(HT):
                        nc.tensor.matmul(
                            sc[:], wv_sb[:, b, h, :], fts[h][:],
                            start=(h == 0), stop=(h == HT - 1))
                    cs = c * TOKC
                    nc.scalar.activation(esc_sb[r:r + 1, cs:cs + TOKC],
                                         sc[r:r + 1, :], Act.Exp)
                    nc.vector.reduce_sum(psum_sb[r:r + 1, c:c + 1],
                                         esc_sb[r:r + 1, cs:cs + TOKC],
                                         axis=AX)
                    nc.vector.tensor_mul(esc_sb[r:r + 1, cs:cs + TOKC],
                                         esc_sb[r:r + 1, cs:cs + TOKC],
                                         vals_sb[r:r + 1, cs:cs + TOKC])

                # softmax denominator; scale in one tensor_scalar at batch end
                nc.vector.reduce_sum(ssum_sb[r:r + 1, :], psum_sb[r:r + 1, :],
                                     axis=AX)
                nc.vector.reciprocal(recip_sb[r:r + 1, :], ssum_sb[r:r + 1, :])
                for g in range(4):
                    gs = g * (NK // 4)
                    nc.vector.tensor_scalar_mul(
                        esc_sb[r:r + 1, gs:gs + NK // 4],
                        esc_sb[r:r + 1, gs:gs + NK // 4],
                        recip_sb[r:r + 1, :])
                    nc.scalar.dma_start(out_ext[b:b + 1, gs:gs + NK // 4],
                                        esc_sb[r:r + 1, gs:gs + NK // 4])

    nc.compile()
    return nc


def shard_inputs(queries, keys, values, W_q, W_k, w_v):
    queries = np.asarray(queries, np.float32)
    keys = np.asarray(keys, np.float32)
    values = np.asarray(values, np.float32)
    W_q = np.asarray(W_q, np.float32)
    W_k = np.asarray(W_k, np.float32)
    w_v = np.asarray(w_v, np.float32)

    def merge_kt(w, ncol):  # [KT*128, ncol] -> [128, KT*ncol] partition-major
        return np.ascontiguousarray(
            w.reshape(KT, 128, ncol).transpose(1, 0, 2).reshape(128, KT * ncol))

    wk2 = merge_kt(W_k, H).astype(ml_dtypes.bfloat16)
    wq2 = merge_kt(W_q, H)
    wv2 = np.zeros((128, B_LOC, HT, 128), np.float32)
    for b in range(B_LOC):
        for h in range(HT):
            wv2[:, b, h, 32 * b] = w_v[h * 128:(h + 1) * 128]
    wv2 = wv2.reshape(128, B_LOC * HT * 128).astype(ml_dtypes.bfloat16)
    in_maps = []
    for i in range(N_CORES):
        b0, b1 = i * B_LOC, (i + 1) * B_LOC
        qT = np.ascontiguousarray(queries[b0:b1, 0, :].T)  # [512, B_LOC]
        in_maps.append({
            "keysT": np.ascontiguousarray(
                keys[b0:b1].transpose(0, 2, 1)).astype(ml_dtypes.bfloat16),
            "queriesT": merge_kt(qT, B_LOC),
            "vals": np.ascontiguousarray(values[b0:b1, :, 0]),
            "wk": wk2, "wq": wq2, "wv": wv2,
        })
    return in_maps


_NC_CACHE = {}


def run(in_maps, trace=False, tmpdir=None):
    from concourse.bass_utils import run_bass_kernel_spmd

    _install_profile_hook()
    try:
        # no artifact bucket inside the container; keep traces local
        import concourse.bass_utils as bu
        bu.upload_artifacts = lambda d: "local://" + d
    except Exception:
        pass
    if "nc" not in _NC_CACHE:
        _NC_CACHE["nc"] = build_nc()
    nc = _NC_CACHE["nc"]
    return run_bass_kernel_spmd(nc, in_maps, core_ids=list(range(N_CORES)),
                                trace=trace, tmpdir=tmpdir)


def kernel(queries, keys, values, W_q, W_k, w_v):
    in_maps = shard_inputs(queries, keys, values, W_q, W_k, w_v)
    res = run(in_maps)
    return np.concatenate([res.results[i]["out"] for i in range(N_CORES)], axis=0)



# revision 6
# speedup vs baseline: 1.1909x; 1.1909x over previous
"""Trainium2 Bass kernel for nn_AdditiveAttention (B=32, NQ=1, NK=4096, D=512, H=256).

Data-parallel over 8 NeuronCores: each core owns 4 batches. Per core:
  kprojT[h, t] = sum_d W_k[d, h] * keys[b, t, d]      (PE, bf16, W_k stationary)
  featT        = tanh(kprojT + qbias_b)               (ACT, bias fused, bf16 out)
  scores       = w_v . featT                          (PE col-tiled matvec: the 4
                                                       batches' scores land on
                                                       rows 0/32/64/96 of ONE
                                                       PSUM tile, concurrently)
  out[b, t]    = softmax_t(scores) * values[b, t]     (exp straight from PSUM with
                                                       accum_out denominators;
                                                       scores are O(4) so no
                                                       max-subtract)

Key points vs the naive layout:
  * qbias (queries @ W_q) is computed on HOST (tiny) - no f32 qproj on device.
  * All softmax-side ops (exp, *values, *1/denom) run on [128, 1024] tiles
    with the 4 batches stacked on partitions 32b - ACT/DVE cost is driven by
    the free-dim size, so processing 4 rows together is 4x cheaper than
    per-batch [1, tok] row ops.
  * The matvec uses tile_position=(0, 32b) col-tiling so the 4 batches'
    matvec matmuls execute concurrently in disjoint 32-col groups of the
    PE array (~4x faster than sequential full-width matmuls).
  * The matvec for chunk c is emitted AFTER kproj of chunk c+1's first batch
    so the last tanh's latency hides under kproj matmuls.
  * Keys arrive as 32 x 0.5MB DMAs so the first kproj can start ~1.5us after
    the first DMA issues; a few bf16 warmup matmuls on memset data bridge the
    preamble and keep the PE HAM clock-gate warming.
"""

import numpy as np
import ml_dtypes

N_CORES = 8
B, NQ, NK, D, H = 32, 1, 4096, 512, 256
B_LOC = B // N_CORES  # 4 batches per core
KT = D // 128         # 4 contraction tiles
HT = H // 128         # 2 hidden tiles
CH = 1024             # token chunk (2 PSUM banks of f32)
NCH = NK // CH        # 4 chunks
N_WARM = 10           # HAM warmup matmuls (bridge until keys arrive)


def _install_profile_hook():
    """Make trace=True / BASS_TRACE=1 usable when the image's antenv lacks
    axon_hooks (degrades silently if anything is missing)."""
    try:
        from antenv import axon_hooks  # noqa: F401
        return
    except ImportError:
        pass
    try:
        import sys
        import types

        import antenv
        from trn_agent_boot.trn_boot import _ntff_profile_via_ctypes

        mod = types.ModuleType("antenv.axon_hooks")
        mod._h = None
        mod.set_axon_ntff_profile_hook = lambda h: setattr(mod, "_h", h)
        mod.get_axon_ntff_profile_hook = lambda: mod._h
        antenv.axon_hooks = mod
        sys.modules["antenv.axon_hooks"] = mod
        mod._h = _ntff_profile_via_ctypes("/opt/axon/libaxon_pjrt.so")
    except Exception:
        pass


def build_nc():
    import concourse.tile as tile
    from concourse import bacc, mybir

    f32 = mybir.dt.float32
    f16 = mybir.dt.bfloat16
    Act = mybir.ActivationFunctionType
    AX = mybir.AxisListType.X

    nc = bacc.Bacc("TRN2", target_bir_lowering=False, debug=False,
                   num_devices=N_CORES)

    keysT_ext = nc.dram_tensor("keysT", [B_LOC, D, NK], f16, kind="ExternalInput")
    qbias_ext = nc.dram_tensor("qbias", [128, HT * B_LOC], f32, kind="ExternalInput")
    vals_ext = nc.dram_tensor("vals", [B_LOC, NK], f32, kind="ExternalInput")
    wk_ext = nc.dram_tensor("wk", [128, KT * H], f16, kind="ExternalInput")
    wv_ext = nc.dram_tensor("wv", [128, B_LOC * HT * 32], f16, kind="ExternalInput")
    out_ext = nc.dram_tensor("out", [B_LOC, NK], f32, kind="ExternalOutput")

    # [B_LOC, D, NK] viewed so one DMA can pull [128 part, KT, ntok]
    keys3d = keysT_ext.ap().rearrange("b (k p) n -> b k p n", p=128)

    with tile.TileContext(nc) as tc:
        with (
            tc.tile_pool(name="keys", bufs=16) as keys_pool,
            tc.tile_pool(name="feat", bufs=12) as feat_pool,
            tc.tile_pool(name="static", bufs=1) as st,
            tc.tile_pool(name="kp", bufs=2, space="PSUM") as kp_pool,
            tc.tile_pool(name="sc", bufs=2, space="PSUM") as sc_pool,
        ):
            # ---- HAM warmup on memset data: PE activity needs no DMA, so
            # the clock-gate starts warming before the first real matmul ----
            wtile = st.tile([128, 256], f16, tag="warm_in")
            nc.vector.memset(wtile[:], 1.0)
            warm_ps = kp_pool.tile([128, CH], f32, tag="kp")
            for w in range(N_WARM):
                nc.tensor.matmul(warm_ps[:, 0:256], wtile[:, 0:128], wtile[:],
                                 start=(w == 0), stop=(w == N_WARM - 1))
            warm_out = st.tile([128, 1], f32, tag="warm")
            nc.vector.reduce_max(warm_out[:], warm_ps[:, 0:256], axis=AX)
            # dummy tanh/exp: force the ACT table load (~2.7us) to happen
            # during the ramp instead of before the first real tanh
            dummy_sb = st.tile([128, 1], f32, tag="dummy")
            nc.scalar.activation(dummy_sb[:], wtile[:, 0:1], Act.Tanh)
            nc.scalar.activation(dummy_sb[:], wtile[:, 0:1], Act.Exp)

            # ---- loads: keys chunks on the sync HWDGE queue (16 engines);
            # small weights ride the scalar queue in parallel; vals rows on
            # the gpsimd queue so nothing queues behind the keys stream ----
            kt_tiles = {}
            for c in range(NCH):
                for b in range(B_LOC):
                    for j in range(2):
                        t = keys_pool.tile([128, KT, 512], f16, tag="kt")
                        s0 = c * CH + j * 512
                        nc.sync.dma_start(
                            t[:], keys3d[b, :, :, s0:s0 + 512]
                            .rearrange("k p n -> p k n"))
                        kt_tiles[(b, c, j)] = t
            wk_sb = st.tile([128, KT, H], f16, tag="wk")
            nc.scalar.dma_start(wk_sb[:], wk_ext.ap())
            qbias_sb = st.tile([128, HT, B_LOC], f32, tag="qbias")
            nc.scalar.dma_start(qbias_sb[:], qbias_ext.ap())
            # w_v per (b, h) as a [128, 32] stationary with the vector in
            # group-col 0, so batch b's scores land on PSUM partition 32*b
            wv_sb = st.tile([128, B_LOC, HT, 32], f16, tag="wv")
            nc.scalar.dma_start(wv_sb[:], wv_ext.ap())
            vals_sb = st.tile([128, NK], f32, tag="vals")
            nc.gpsimd.memset(vals_sb[:], 0.0)
            for b in range(B_LOC):
                nc.gpsimd.dma_start(vals_sb[32 * b:32 * b + 1, :],
                                    vals_ext[b:b + 1, :])

            # ---- per-core softmax state (batch b on partition 32*b) ----
            esc_sb = st.tile([128, NK], f32, tag="esc")       # exp(scores)*vals
            psum_sb = st.tile([128, NCH], f32, tag="psums")   # per-chunk denoms
            ssum_sb = st.tile([128, 1], f32, tag="ssum")
            recip_sb = st.tile([128, 1], f32, tag="recip")

            fts = {}      # (b, h) -> feat tile of the current chunk
            sc_tiles = {}  # c -> scores PSUM tile

            def emit_kproj(c, b):
                for h in range(HT):
                    ps = kp_pool.tile([128, CH], f32, tag="kp")
                    for j in range(2):
                        src = kt_tiles[(b, c, j)]
                        for k in range(KT):
                            nc.tensor.matmul(
                                ps[:, j * 512:(j + 1) * 512],
                                wk_sb[:, k, h * 128:(h + 1) * 128],
                                src[:, k, :],
                                start=(k == 0), stop=(k == KT - 1),
                            )
                    ft = feat_pool.tile([128, CH], f16, tag="ft")
                    nc.scalar.activation(ft[:], ps[:], Act.Tanh,
                                         bias=qbias_sb[:, h, b:b + 1])
                    fts[(b, h, c)] = ft

            def emit_matvec(c):
                # col-tiled: the 4 batches' matmuls target disjoint 32-col
                # groups of the PE array and run concurrently
                sc = sc_tiles[c]
                for h in range(HT):
                    for j in range(2):
                        for b in range(B_LOC):
                            nc.tensor.matmul(
                                sc[32 * b:32 * b + 32, j * 512:(j + 1) * 512],
                                wv_sb[:, b, h, :],
                                fts[(b, h, c)][:, j * 512:(j + 1) * 512],
                                start=(h == 0), stop=(h == HT - 1),
                                tile_position=(0, 32 * b),
                                skip_group_check=True,
                            )
                cs = c * CH
                nc.scalar.activation(esc_sb[:, cs:cs + CH], sc[:], Act.Exp,
                                     accum_out=psum_sb[:, c:c + 1])
                nc.vector.tensor_mul(esc_sb[:, cs:cs + CH],
                                     esc_sb[:, cs:cs + CH],
                                     vals_sb[:, cs:cs + CH])

            for c in range(NCH):
                sc_tiles[c] = sc_pool.tile([128, CH], f32, tag="sc", name="sc")
                for b in range(B_LOC):
                    emit_kproj(c, b)
                    # defer chunk c-1's matvec until after kproj(c, b0) so
                    # the last tanh's latency hides under kproj matmuls
                    if b == 0 and c > 0:
                        emit_matvec(c - 1)
            emit_matvec(NCH - 1)

            # softmax denominator; scale split across DVE and ACT
            nc.vector.reduce_sum(ssum_sb[:], psum_sb[:], axis=AX)
            nc.vector.reciprocal(recip_sb[:], ssum_sb[:])
            for g in range(NCH):
                gs = g * CH
                if g % 2 == 0:
                    nc.vector.tensor_scalar_mul(
                        esc_sb[:, gs:gs + CH], esc_sb[:, gs:gs + CH],
                        recip_sb[:])
                else:
                    nc.scalar.mul(esc_sb[:, gs:gs + CH], esc_sb[:, gs:gs + CH],
                                  recip_sb[:])
            out_engines = [nc.sync, nc.scalar, nc.gpsimd, nc.sync]
            for b in range(B_LOC):
                out_engines[b].dma_start(out_ext[b:b + 1, :],
                                         esc_sb[32 * b:32 * b + 1, :])

    nc.compile()
    return nc


def shard_inputs(queries, keys, values, W_q, W_k, w_v):
    queries = np.asarray(queries, np.float32)
    keys = np.asarray(keys, np.float32)
    values = np.asarray(values, np.float32)
    W_q = np.asarray(W_q, np.float64)
    W_k = np.asarray(W_k, np.float32)
    w_v = np.asarray(w_v, np.float32)

    def merge_kt(w, ncol):  # [KT*128, ncol] -> [128, KT*ncol] partition-major
        return np.ascontiguousarray(
            w.reshape(KT, 128, ncol).transpose(1, 0, 2).reshape(128, KT * ncol))

    wk2 = merge_kt(W_k, H).astype(ml_dtypes.bfloat16)
    wv2 = np.zeros((128, B_LOC, HT, 32), np.float32)
    for b in range(B_LOC):
        for h in range(HT):
            wv2[:, b, h, 0] = w_v[h * 128:(h + 1) * 128]
    wv2 = wv2.reshape(128, B_LOC * HT * 32).astype(ml_dtypes.bfloat16)

    # qbias on host (tiny): [B, H] = queries @ W_q, exact in f64
    qb_all = (queries[:, 0, :].astype(np.float64) @ W_q).astype(np.float32)

    in_maps = []
    for i in range(N_CORES):
        b0, b1 = i * B_LOC, (i + 1) * B_LOC
        qb = np.zeros((128, HT, B_LOC), np.float32)
        for b in range(B_LOC):
            for h in range(HT):
                qb[:, h, b] = qb_all[b0 + b, h * 128:(h + 1) * 128]
        in_maps.append({
            "keysT": np.ascontiguousarray(
                keys[b0:b1].transpose(0, 2, 1)).astype(ml_dtypes.bfloat16),
            "qbias": qb.reshape(128, HT * B_LOC),
            "vals": np.ascontiguousarray(values[b0:b1, :, 0]),
            "wk": wk2, "wv": wv2,
        })
    return in_maps


_NC_CACHE = {}


def run(in_maps, trace=False, tmpdir=None):
    from concourse.bass_utils import run_bass_kernel_spmd

    _install_profile_hook()
    try:
        # no artifact bucket inside the container; keep traces local
        import concourse.bass_utils as bu
        bu.upload_artifacts = lambda d: "local://" + d
    except Exception:
        pass
    if "nc" not in _NC_CACHE:
        _NC_CACHE["nc"] = build_nc()
    nc = _NC_CACHE["nc"]
    return run_bass_kernel_spmd(nc, in_maps, core_ids=list(range(N_CORES)),
                                trace=trace, tmpdir=tmpdir)


def kernel(queries, keys, values, W_q, W_k, w_v):
    in_maps = shard_inputs(queries, keys, values, W_q, W_k, w_v)
    res = run(in_maps)
    return np.concatenate([res.results[i]["out"] for i in range(N_CORES)], axis=0)
